# revision 35
# baseline (speedup 1.0000x reference)
"""Differential attention (B=2, N=2048, D=1024, H=8, HEAD_DIM=128) on 8 trn2
NeuronCores. Head-parallel: core h computes head h end-to-end, then an
AllToAll re-shards heads -> token blocks for the output projection, so each
core emits one 512-token slice of the final output (no cross-core reduction).

Layout convention on device: activations are kept feature-major ("transposed",
[feature, token]) so that matmuls contract over the partition dim without any
on-chip transposition of x. The host supplies x pre-transposed and transposes
the output back.
"""

import numpy as np

import concourse.bass as bass
import concourse.mybir as mybir
import concourse.tile as tile
from concourse.bass_utils import run_bass_kernel_spmd
from concourse.masks import make_identity
from concourse.vector_clock import ScopedClock

# ---------------------------------------------------------------- constants
B, N, D = 2, 2048, 1024
H, HD = 8, 128
DQK = HD // 2
PROJ = H * HD
T = B * N  # 4096 flattened tokens
NCORES = 8
TBLK = T // NCORES  # 512 tokens per core for the output projection
LAMBDA_INIT = 0.8 - 0.6 * float(np.exp(-0.3 * 12))
SCALE = DQK ** -0.5
EPS = 1e-6

KB = N // 128  # 16 key chunks per batch
QB = N // 512  # 4 query blocks of 512 per batch

FP = mybir.dt.float32
FR = mybir.dt.float32r
BF = mybir.dt.bfloat16


# ------------------------------------------------- walrus drain workaround
# This container's walrus rejects Drain instructions carrying >1 sync wait
# ("Too many sync wait commands"). Split the TileContext tail drain into one
# Drain per wait condition.
def _split_waits(nc, inst, max_waits=1):
    si = inst.ins.sync_info
    if si is None:
        return
    waits = list(si.on_wait)
    if len(waits) <= max_waits:
        return
    si.on_wait = waits[:max_waits]
    for w in waits[max_waits:]:
        d2 = nc.sync.drain(fusable=False)
        si2 = d2.ins.sync_info
        if si2 is None:
            d2.ins.sync_info = mybir.SyncInfo(on_wait=[w], on_update=[])
        else:
            si2.on_wait = [w]


def _split_all_multiwaits(nc, max_waits=1):
    """walrus here allows only `max_waits` sync-wait per instruction. Hoist
    extra waits onto fresh NoOps inserted just before the instruction on the
    same engine (engines dispatch in order, so semantics are preserved)."""
    uid = 0
    for fn in nc.m.functions:
        for bb in fn.blocks:
            il = bb.instructions
            changed = False
            out = []
            for inst in il:
                si = inst.sync_info
                waits = list(si.on_wait) if si is not None else []
                if len(waits) > max_waits:
                    for w in waits[:-max_waits]:
                        ev = mybir.InstEventSemaphore(
                            name=f"waitsplit_{uid}",
                            sync_info=mybir.SyncInfo(on_wait=[w], on_update=[]),
                            engine=inst.engine,
                        )
                        uid += 1
                        out.append(ev)
                    si.on_wait = waits[-max_waits:]
                    if inst.sync_info is not si:
                        inst.sync_info = si
                    changed = True
                out.append(inst)
            if changed:
                bb.instructions = out


def _patched_drain_and_barrier(self, tick_clock, wait_clock):
    nc = self.nc
    drain_inst = nc.sync.drain(fusable=False)
    wait_clock.add_sem_waits(
        drain_inst.ins, ScopedClock({None: tick_clock.global_clock})
    )
    _split_waits(nc, drain_inst)
    nc.all_engine_barrier()
    assert self.sems is not None
    popped = nc._tile_sem_poison_stack.pop()
    assert popped is self._sem_poison
    nc.clear_and_free_semaphores(list(self.sems.allocated().values()))
    nc.all_engine_barrier()


tile.TileContext._drain_and_barrier = _patched_drain_and_barrier


# ---------------------------------------------------------------- program
def build_program(dbg=False, reps=1, skip_cc=False):
    nc = bass.Bass(
        "TRN2",
        target_bir_lowering=False,
        debug=False,
        enable_asserts=True,
        num_devices=NCORES,
    )

    DC = D // 128  # contraction chunks for the qkv projection
    xT = nc.dram_tensor("xT", [D, T], BF, kind="ExternalInput")
    wq = nc.dram_tensor("wq", [128, DC * HD], BF, kind="ExternalInput")
    wk = nc.dram_tensor("wk", [128, DC * HD], BF, kind="ExternalInput")
    wv = nc.dram_tensor("wv", [128, DC * HD], BF, kind="ExternalInput")
    wp = nc.dram_tensor("wp", [128, H * D], BF, kind="ExternalInput")
    lam = nc.dram_tensor("lam", [128, 1], FP, kind="ExternalInput")
    # token-major output: row = local token, col = model feature
    yT = nc.dram_tensor("yT", [TBLK, D], FP, kind="ExternalOutput")
    if dbg:
        d_qT = nc.dram_tensor("d_qT", [128, T], BF, kind="ExternalOutput")
        d_kT = nc.dram_tensor("d_kT", [128, T], BF, kind="ExternalOutput")
        d_va = nc.dram_tensor("d_va", [128, B * KB, HD + 1], BF, kind="ExternalOutput")
        d_U = nc.dram_tensor("d_U", [B * QB * 4, 128, 2 * (HD + 1)], FP, kind="ExternalOutput")

    with tile.TileContext(nc, num_cores=NCORES) as tc:
        with (
            tc.tile_pool(name="consts", bufs=1) as consts,
            tc.tile_pool(name="dram", bufs=1, space="DRAM") as dram,
        ):
            ident = consts.tile([128, 128], FP)
            make_identity(nc, ident)
            # bf16 identity for the v transposes — bf16 LDWEIGHTS+MATMUL run
            # at 1 cyc/row vs fp32's 1.5-2, and v is stored bf16 anyway
            identb = consts.tile([128, 128], BF)
            nc.vector.tensor_copy(identb[:], ident[:])
            lam_sb = consts.tile([128, 1], FP)
            nc.sync.dma_start(lam_sb[:], lam[:])

            wq_sb = consts.tile([128, DC, HD], BF)
            wk_sb = consts.tile([128, DC, HD], BF)
            wv_sb = consts.tile([128, DC, HD], BF)
            # qkv weights stay on the SP ring ahead of the x tiles (the first
            # matmuls need them); only the big wp + lam ride the ACT ring
            for w_dram, w_sb in ((wq, wq_sb), (wk, wk_sb), (wv, wv_sb)):
                nc.sync.dma_start(w_sb[:], w_dram.rearrange("p (c m) -> p c m", c=DC))
            # wp is 2MB — issue from the scalar-engine HWDGE ring so it never
            # delays the x tiles feeding the first qkv matmuls on the SP ring
            wp_sb = consts.tile([128, H, D], BF)
            nc.scalar.dma_start(wp_sb[:], wp.rearrange("p (h m) -> p h m", h=H))

            qT_b = [consts.tile([128, N], BF, name=f"qT_{b}") for b in range(B)]
            kT_b = [consts.tile([128, N], BF, name=f"kT_{b}") for b in range(B)]
            # v, per (batch, key-chunk): [key, head_dim] plus a ones column
            # (col 128) so the PV matmul also accumulates the softmax denom.
            va_b = [consts.tile([128, KB, HD + 1], BF, name=f"va_{b}") for b in range(B)]
            for b in range(B):
                nc.vector.memset(va_b[b][:, :, HD : HD + 1], 1.0)

            # A2A re-shard, chunked: collective group g carries the blocks in
            # CC_GROUPS[g], firing as soon as its last block's epilogue lands,
            # so early exchanges overlap later blocks' attention.  The final
            # two blocks get their own single-block groups: tb6's exchange
            # overlaps tb7's attention and only tb7's 128KB exchange remains
            # on the tail.
            # Layout: in_g[c] = this head's [hd, len(blocks) x 64 tokens]
            # destined for core c (tokens c*64..c*64+64 of each block).
            NCHUNK = QB * B // 2
            CC_GROUPS = [(0, 1), (2, 3), (4, 5), (6,), (7,)]
            CC_OF_TB = {tb: g for g, blks in enumerate(CC_GROUPS) for tb in blks}
            a2a_in = [
                dram.tile([NCORES, 128, 64 * len(blks)], BF, name=f"a2a_in_{g}")
                for g, blks in enumerate(CC_GROUPS)
            ]
            a2a_out = [
                dram.tile([NCORES, 128, 64 * len(blks)], BF, name=f"a2a_out_{g}")
                for g, blks in enumerate(CC_GROUPS)
            ]
            # warmup collective: syncs the 8 cores right at kernel start and
            # absorbs the first-collective setup cost while the PE is still
            # waiting on the x DMAs — the first real A2A then runs at the
            # steady-state ~7us instead of ~20us.
            if not skip_cc:
                warm_in = dram.tile([NCORES, 128, 128], BF, name="warm_in")
                warm_out = dram.tile([NCORES, 128, 128], BF, name="warm_out")
                nc.gpsimd.collective_compute(
                    "AllToAll",
                    mybir.AluOpType.bypass,
                    replica_groups=[list(range(NCORES))],
                    ins=[warm_in.opt()],
                    outs=[warm_out.opt()],
                )
            # gathered activations for the output projection, one tile per
            # chunk so the out-proj chunk j only depends on chunk j's A2A
            aa = [
                consts.tile([128, H, 128], BF, name=f"aa_{j}") for j in range(NCHUNK)
            ]

            for rep in range(reps):
                # ---------------- phase A: qkv projection (feature-major) ----
                with (
                    tc.tile_pool(name="xa", bufs=2) as xa,
                    tc.tile_pool(name="pa", bufs=2, space="PSUM") as pa,
                    tc.tile_pool(name="sa", bufs=2) as sa,
                ):
                    xT_view = xT.rearrange("(c p) t -> p c t", p=128)
                    for tp in range(T // 1024):  # 1024-token pairs
                        b = tp // 2
                        ts2 = slice(tp * 1024, (tp + 1) * 1024)
                        xx = [
                            xa.tile([128, 1024], BF, tag=f"xx{c}", name=f"xx_{tp}_{c}")
                            for c in range(DC)
                        ]
                        for c in range(DC):
                            nc.sync.dma_start(xx[c][:], xT_view[:, c, ts2])
                        for half in range(2):
                            tb = tp * 2 + half
                            hs = slice(half * 512, (half + 1) * 512)
                            bs = slice((tb % QB) * 512, (tb % QB + 1) * 512)
                            qps = pa.tile([128, 512], FP, tag="qps", name=f"qps_{tb}")
                            kps = pa.tile([128, 512], FP, tag="kps", name=f"kps_{tb}")
                            vps = pa.tile([128, 512], FP, tag="vps", name=f"vps_{tb}")
                            for c in range(DC):
                                f = dict(start=(c == 0), stop=(c == DC - 1))
                                nc.tensor.matmul(qps[:], wq_sb[:, c, :], xx[c][:, hs], **f)
                                nc.tensor.matmul(kps[:], wk_sb[:, c, :], xx[c][:, hs], **f)
                                nc.tensor.matmul(vps[:], wv_sb[:, c, :], xx[c][:, hs], **f)
                            nc.vector.tensor_copy(qT_b[b][:, bs], qps[:])
                            nc.vector.tensor_copy(kT_b[b][:, bs], kps[:])

                            # v must be token-major for the PV matmul:
                            # PE-transpose 128x128 chunks of vT (bf16: 1
                            # cyc/row and single-pass LDWEIGHTS, vs fp32's 2x)
                            vT = sa.tile([128, 512], BF, tag="vT", name=f"vT_{tb}")
                            nc.vector.tensor_copy(vT[:], vps[:])
                            for j in range(4):
                                kb = (tb % QB) * 4 + j
                                vtp = pa.tile([128, 128], BF, tag="vtp",
                                              name=f"vtp_{tb}_{j}")
                                nc.tensor.transpose(
                                    vtp[:], vT[:, j * 128 : (j + 1) * 128], identb[:]
                                )
                                nc.vector.tensor_copy(va_b[b][:, kb, 0:HD], vtp[:])

                # ---------------- phase B: differential attention ------------
                # Per-block epilogues are DEFERRED into the next block's kb
                # loop: the ACT queue is a strict FIFO, so putting the small
                # Ln/Exp behind a few already-ready big exps removes the
                # per-block ACT head-of-line stall.  A2A chunks fire as their
                # two blocks complete; out-proj chunks run inline on borrowed
                # U-pool PSUM slots, so only the last chunk sits on the tail.
                with (
                    tc.tile_pool(name="ps", bufs=1, space="PSUM") as ps,
                    tc.tile_pool(name="pu", bufs=1, space="PSUM") as pu,
                    tc.tile_pool(name="pp", bufs=8) as pp,
                    tc.tile_pool(name="se", bufs=2) as se,
                    tc.tile_pool(name="so", bufs=8) as so,
                ):
                    def emit_epilogue(tb, Usb):
                        gch = CC_OF_TB[tb]
                        blks = CC_GROUPS[gch]
                        slot = blks.index(tb)
                        # Mid-stream blocks batch the 4 subs' ms into one
                        # [128, 4] tile so a single Ln + Exp pair serves the
                        # whole block (ACT is the phase-B rate limiter; keep it
                        # on big exps).  The LAST block instead pipelines fully
                        # per-sub — ACT is idle by then, and each sub's
                        # transpose+write starts without waiting for sub3's ms,
                        # shortening the tail's epilogue->collective chain.
                        last = tb == B * QB - 1
                        ms4 = se.tile([128, 4], FP, tag="ms4", name=f"ms4_{tb}")
                        rs4 = se.tile([128, 4], FP, tag="rs4", name=f"rs4_{tb}")

                        def finish_sub(sub, od):
                            # on = od * rsqrt(ms); transpose via the DMA xbar
                            # (2-byte dtype) so the PE queue is never blocked
                            # on the epilogue; then scatter the two 64-token
                            # halves: block-local tokens sub*128+t go to core
                            # c = 2*sub + t//64, at slot `slot` of group gch.
                            # The last block's DMAs sit on the critical tail —
                            # spread them over the SP and (by then idle) ACT
                            # HWDGE rings to halve their serial latency.
                            eng = nc.scalar if (last and sub % 2) else nc.sync
                            on = se.tile([128, 128], BF, tag="on", bufs=4,
                                         name=f"on_{tb}_{sub}")
                            nc.vector.tensor_scalar_mul(
                                on[:], od[:], rs4[:, sub : sub + 1]
                            )
                            onT = so.tile([128, 128], BF, tag="onT")
                            eng.dma_start_transpose(onT[:], on[:])
                            eng.dma_start(
                                a2a_in[gch][2 * sub, :, slot * 64 : slot * 64 + 64],
                                onT[:, 0:64],
                            )
                            eng.dma_start(
                                a2a_in[gch][2 * sub + 1, :, slot * 64 : slot * 64 + 64],
                                onT[:, 64:128],
                            )

                        ods = []
                        for sub in range(4):
                            u = Usb[sub]
                            if dbg:
                                ud = se.tile([128, 2 * (HD + 1)], FP, tag="ud")
                                nc.vector.tensor_copy(ud[:], u[:])
                                nc.sync.dma_start(d_U[tb * 4 + sub], ud[:])
                            r1 = se.tile([128, 1], FP, tag="r1")
                            r2 = se.tile([128, 1], FP, tag="r2")
                            nc.vector.reciprocal(r1[:], u[:, HD : HD + 1])
                            nc.vector.reciprocal(r2[:], u[:, 2 * HD + 1 : 2 * HD + 2])
                            r2l = se.tile([128, 1], FP, tag="r2l")
                            nc.vector.tensor_mul(r2l[:], r2[:], lam_sb[:])
                            t1 = se.tile([128, 128], FP, tag="t1")
                            t2 = se.tile([128, 128], FP, tag="t2")
                            nc.vector.tensor_scalar_mul(t1[:], u[:, 0:HD], r1[:])
                            nc.vector.tensor_scalar_mul(
                                t2[:], u[:, HD + 1 : 2 * HD + 1], r2l[:]
                            )
                            od = se.tile([128, 128], FP, tag=f"od{sub}")
                            nc.vector.tensor_sub(od[:], t1[:], t2[:])
                            ods.append(od)
                            # ms = EPS + mean(od^2)
                            sq = se.tile([128, 128], FP, tag="sq")
                            nc.vector.tensor_mul(sq[:], od[:], od[:])
                            ssum = se.tile([128, 1], FP, tag="ssum")
                            nc.vector.tensor_reduce(
                                ssum[:], sq[:], mybir.AxisListType.X,
                                mybir.AluOpType.add,
                            )
                            nc.vector.tensor_scalar(
                                ms4[:, sub : sub + 1], ssum[:], 1.0 / HD, EPS,
                                mybir.AluOpType.mult, mybir.AluOpType.add,
                            )
                            if last:
                                nc.scalar.activation(
                                    rs4[:, sub : sub + 1], ms4[:, sub : sub + 1],
                                    mybir.ActivationFunctionType.Ln,
                                )
                                nc.scalar.activation(
                                    rs4[:, sub : sub + 1], rs4[:, sub : sub + 1],
                                    mybir.ActivationFunctionType.Exp,
                                    scale=-0.5,
                                )
                                finish_sub(sub, od)
                        if not last:
                            # rsqrt(ms) = exp(-0.5*ln(ms)); Log and Exp share
                            # one ACT table set, so no table-switch thrash
                            # against the attention exps.
                            rt4 = se.tile([128, 4], FP, tag="rt4", name=f"rt4_{tb}")
                            nc.scalar.activation(
                                rt4[:], ms4[:], mybir.ActivationFunctionType.Ln
                            )
                            nc.scalar.activation(
                                rs4[:], rt4[:], mybir.ActivationFunctionType.Exp,
                                scale=-0.5,
                            )
                            for sub in range(4):
                                finish_sub(sub, ods[sub])
                        # after the group's last block, fire its A2A and
                        # prefetch the received slices for the out-proj
                        if slot == len(blks) - 1:
                            if skip_cc:
                                nc.sync.dma_start(a2a_out[gch][:], a2a_in[gch][:])
                            else:
                                nc.gpsimd.collective_compute(
                                    "AllToAll",
                                    mybir.AluOpType.bypass,
                                    replica_groups=[list(range(NCORES))],
                                    ins=[a2a_in[gch].opt()],
                                    outs=[a2a_out[gch].opt()],
                                )
                            # aa reads wait on the collective — issue them
                            # from the (idle) gpsimd queue so they never
                            # head-of-line-block the epilogue's sync-queue
                            # transposes and a2a_in writes.  For the final
                            # group use the ACT HWDGE ring instead: it is idle
                            # by then and kicks in ~0.1us vs gpsimd's ~0.7us.
                            jaa = min(gch, NCHUNK - 1)
                            c0 = (blks[0] - 2 * jaa) * 64
                            w = 64 * len(blks)
                            for s in range(NCORES):
                                if last:
                                    # both HWDGE rings are idle on the tail —
                                    # alternate to halve the serial latency
                                    eng2 = nc.scalar if s % 2 else nc.sync
                                    eng2.dma_start(
                                        aa[jaa][:, s, c0 : c0 + w], a2a_out[gch][s]
                                    )
                                else:
                                    nc.gpsimd.dma_start(
                                        aa[jaa][:, s, c0 : c0 + w], a2a_out[gch][s]
                                    )

                    def emit_outproj_chunk(j, tok0=0, ntok=128):
                        # out-proj for chunk j's tokens [tok0, tok0+ntok),
                        # token-stationary: lhsT = aa_j slice [hd, tok]
                        # (stationary), rhs = wp [hd, 512 feats] (moving)
                        # -> [ntok, 512 feats], accumulated over the 8 heads.
                        # PSUM slots borrowed from the U rotation (all 4 U
                        # banks are free right after the u_sb copies).
                        for of in range(2):
                            yps = pu.tile([128, 512], FP, tag="U", bufs=4,
                                          name=f"yps_{rep}_{j}_{tok0}_{of}")
                            for hh in range(H):
                                nc.tensor.matmul(
                                    yps[0:ntok, :],
                                    aa[j][:, hh, tok0 : tok0 + ntok],
                                    wp_sb[:, hh, of * 512 : (of + 1) * 512],
                                    start=(hh == 0),
                                    stop=(hh == H - 1),
                                )
                            yo = se.tile([128, 512], FP, tag="yo",
                                         name=f"yo_{rep}_{j}_{tok0}_{of}")
                            nc.vector.tensor_copy(yo[0:ntok, :], yps[0:ntok, :])
                            eng = nc.scalar if j == NCHUNK - 1 else nc.sync
                            eng.dma_start(
                                yT[j * 128 + tok0 : j * 128 + tok0 + ntok,
                                   of * 512 : (of + 1) * 512],
                                yo[0:ntok, :],
                            )

                    pending = None  # deferred epilogue: (tb, Usb)
                    for b in range(B):
                        for qb in range(QB):
                            tb = b * QB + qb  # global 512-token block id
                            qs = slice(qb * 512, (qb + 1) * 512)
                            # padded to 512 cols (exactly one PSUM bank) so the
                            # tag ring slots interchange with the borrowed
                            # out-proj accumulators
                            U = [
                                pu.tile([128, 512], FP, tag="U", bufs=4,
                                        name=f"U_{rep}_{tb}_{i}")
                                for i in range(4)
                            ]
                            for kb in range(KB):
                                if kb == 4 and pending is not None:
                                    emit_epilogue(*pending)
                                    pending = None
                                # chunk 2's out-proj runs mid-way through the
                                # last block so its DVE copies and DMAs never
                                # contend with tb7's tail epilogue
                                if kb == 10 and tb == 7:
                                    emit_outproj_chunk(2)
                                ks = slice(kb * 128, (kb + 1) * 128)
                                s12 = ps.tile([128, 1024], FP, tag="s12", bufs=2)
                                # S^T tiles [key, query] for both q/k streams,
                                # row-packed on the PE (K=64 each).
                                nc.tensor.matmul(
                                    s12[:, 0:512],
                                    kT_b[b][0:64, ks],
                                    qT_b[b][0:64, qs],
                                    start=True, stop=True,
                                )
                                nc.tensor.matmul(
                                    s12[:, 512:1024],
                                    kT_b[b][64:128, ks],
                                    qT_b[b][64:128, qs],
                                    start=True, stop=True,
                                )
                                p12 = pp.tile([128, 1024], BF)
                                nc.scalar.activation(
                                    p12[:], s12[:], mybir.ActivationFunctionType.Exp
                                )
                                vak = va_b[b][:, kb, :]
                                for s in range(2):
                                    for sub in range(4):
                                        # start=True clears has_written for the
                                        # whole PSUM bank, so only the very first
                                        # matmul touching this U bank may set it.
                                        nc.tensor.matmul(
                                            U[sub][:, s * (HD + 1) : (s + 1) * (HD + 1)],
                                            p12[:, s * 512 + sub * 128 : s * 512 + (sub + 1) * 128],
                                            vak,
                                            start=(kb == 0 and s == 0),
                                            stop=(kb == KB - 1 and s == 1),
                                        )
                            # copy U banks to SBUF so the PSUM slots free
                            # immediately — the next block's PV matmuls (and
                            # borrowed out-proj slots) reuse them.
                            Usb = []
                            for sub in range(4):
                                u_sb = se.tile(
                                    [128, 2 * (HD + 1)], FP, tag=f"usb{sub}",
                                    name=f"usb_{rep}_{tb}_{sub}",
                                )
                                nc.vector.tensor_copy(u_sb[:], U[sub][:, 0 : 2 * (HD + 1)])
                                Usb.append(u_sb)
                            pending = (tb, Usb)
                            # out-proj chunks as late as safely possible: the
                            # chunk's A2A completes comfortably before the PE
                            # reaches these matmuls, so they never stall the
                            # attention stream behind them in the PE queue.
                            if tb == 5:
                                emit_outproj_chunk(0)
                            elif tb == 6:
                                emit_outproj_chunk(1)
                    # chunk 3 in two column-halves: the tb6 half's data was
                    # exchanged while tb7 was still computing, so its matmuls
                    # fill the otherwise-idle PE during the final epilogue —
                    # only tb7's own 64 tokens remain on the tail.
                    emit_epilogue(*pending)
                    emit_outproj_chunk(NCHUNK - 1, 0, 64)
                    emit_outproj_chunk(NCHUNK - 1, 64, 64)

                if dbg:
                    for b in range(B):
                        nc.sync.dma_start(d_qT[:, b * N : (b + 1) * N], qT_b[b][:])
                        nc.sync.dma_start(d_kT[:, b * N : (b + 1) * N], kT_b[b][:])
                        nc.sync.dma_start(d_va[:, b * KB : (b + 1) * KB, :], va_b[b][:])

    _split_all_multiwaits(nc)
    return nc


_PROGRAM = None


def _get_program():
    global _PROGRAM
    if _PROGRAM is None:
        _PROGRAM = build_program()
    return _PROGRAM


# ---------------------------------------------------------------- host side
def _prep_in_maps(x, w_qkv, w_proj, lambda_q1, lambda_k1, lambda_q2, lambda_k2,
                  rms_weight):
    import ml_dtypes

    x = np.asarray(x, dtype=np.float32)
    w_qkv = np.asarray(w_qkv, dtype=np.float32)
    w_proj = np.asarray(w_proj, dtype=np.float32)
    xT = np.ascontiguousarray(x.reshape(T, D).T).astype(ml_dtypes.bfloat16)
    lam_val = (
        float(np.exp(np.sum(np.asarray(lambda_q1, np.float64) * np.asarray(lambda_k1, np.float64))))
        - float(np.exp(np.sum(np.asarray(lambda_q2, np.float64) * np.asarray(lambda_k2, np.float64))))
        + LAMBDA_INIT
    )
    lam_arr = np.full((128, 1), lam_val, dtype=np.float32)
    # fold rms_weight and (1 - lambda_init) into the output projection rows
    rw = np.asarray(rms_weight, np.float32)
    wp_full = np.ascontiguousarray(
        w_proj * np.tile(rw, H)[:, None] * np.float32(1.0 - LAMBDA_INIT)
    )
    # device-friendly layouts: weights arranged so each DMA descriptor is a
    # long contiguous run per partition
    def chunked(w):  # [D, HD] -> [128, DC*HD] with [p, c*HD+m] = w[c*128+p, m]
        dc = D // 128
        return np.ascontiguousarray(
            w.reshape(dc, 128, HD).transpose(1, 0, 2).reshape(128, dc * HD)
        ).astype(ml_dtypes.bfloat16)

    wp_dev = np.ascontiguousarray(
        wp_full.reshape(H, 128, D).transpose(1, 0, 2).reshape(128, H * D)
    ).astype(ml_dtypes.bfloat16)
    in_maps = []
    for h in range(NCORES):
        hs = slice(h * HD, (h + 1) * HD)
        in_maps.append(
            {
                "xT": xT,
                "wq": chunked(np.ascontiguousarray(w_qkv[:, hs]) * np.float32(SCALE)),
                "wk": chunked(w_qkv[:, PROJ + h * HD : PROJ + (h + 1) * HD]),
                "wv": chunked(w_qkv[:, 2 * PROJ + h * HD : 2 * PROJ + (h + 1) * HD]),
                "wp": wp_dev,
                "lam": lam_arr,
            }
        )
    return in_maps


def _assemble(results):
    # core c's yT row j*128 + slot*64 + t holds global token
    # (2j+slot)*512 + c*64 + t  (chunked-A2A token interleave)
    y = np.empty((T, D), dtype=np.float32)
    for c in range(NCORES):
        yTc = results[c]["yT"]
        for tb in range(B * QB):
            jch, slot = tb // 2, tb % 2
            y[tb * 512 + c * 64 : tb * 512 + (c + 1) * 64, :] = (
                yTc[jch * 128 + slot * 64 : jch * 128 + slot * 64 + 64, :]
            )
    return y.reshape(B, N, D)


def kernel(x, w_qkv, w_proj, lambda_q1, lambda_k1, lambda_q2, lambda_k2,
           rms_weight):
    nc = _get_program()
    in_maps = _prep_in_maps(
        x, w_qkv, w_proj, lambda_q1, lambda_k1, lambda_q2, lambda_k2, rms_weight
    )
    res = run_bass_kernel_spmd(nc, in_maps, list(range(NCORES)))
    return _assemble(res.results)

